# revision 11
# baseline (speedup 1.0000x reference)
"""Trainium2 Bass kernel for nn_ContrastiveLoss (B=4096, D=512, 8 cores).

Strategy v4 (symmetric-triangle, pad-free column grouping):
  - Host l2-normalizes [emb_i; emb_j] -> reps [8192, 512] (fp32), quantizes
    to fp8 e4m3, and packs per-core SBUF layouts.
  - The 8192x8192 similarity matrix is a 16x16 grid of 512x512 cells; only
    the 136 upper-triangle cells are computed (sim is symmetric).  Work is
    split into "units" = (cell, 128-row sub-slice): 544 units.  A device
    tile [128 x 2048] packs 4 units that share one 512-column block (grid
    column j has 4(j+1) units -> exactly j+1 tiles, no padding): 136 tiles
    total = 17 per core, perfectly balanced.
  - Per tile: fp8 DoubleRow matmuls (K=256/instr) -> 4-bank PSUM, ScalarE
    exp(5x) reading PSUM directly, fp8 result DMAed out on the sync queue.
    qt streams on the gpsimd queue, xt on the vector queue; dummy warm-up
    matmuls keep the PE HAM clock-gate at 8/8 before real work arrives.
  - Host reduces: row sums of each cell feed its grid-row block, column
    sums of off-diagonal cells feed the transposed block, then the known
    fp8 diagonal term is subtracted and the host-computed fu scalar and
    positive-pair terms are added.

exp(5*sim) lies in [e^-1.5, e^5] ~ [0.2, 149]: inside fp8 e4m3 normal
range.  fp8 quantization of inputs+outputs yields ~5e-5 final loss error
vs the 2e-2 gate.
"""

import numpy as np

import concourse.bass as bass
import concourse.mybir as mybir
import concourse.tile as tile
from concourse import bacc

f32 = mybir.dt.float32
fp8 = mybir.dt.float8e4
AF = mybir.ActivationFunctionType

P = 128
TEMP = 0.2
INV_T = 1.0 / TEMP  # 5.0
FP8_NP = mybir.dt.np(fp8)

B, D = 4096, 512
TWO_N = 2 * B           # 8192
KT = D // P             # 4 k-subtiles of 128
CHUNK = 2048            # ACT/psum tile free size (4 PSUM banks)
CELL = 512              # grid cell edge
G = TWO_N // CELL       # 16x16 grid
N_TILES = 17            # tiles per core (136 total / 8)
W = N_TILES * CELL      # 8704 packed columns (both qt and xt)


def _tiles():
    """136 (col_block_j, [(i,s) x 4]) tiles covering the upper triangle."""
    out = []
    for j in range(G):
        units = [(i, s) for i in range(j + 1) for s in range(4)]
        for t in range(0, len(units), 4):
            out.append((j, units[t:t + 4]))
    assert len(out) == 8 * N_TILES
    return out


TILES = _tiles()


def build_nc():
    """SPMD program: tile t computes, for g in 0..3,
    exp(5 * q[(4t+g)*128 : +128] @ x[t*512 : +512]) -> eout[:, t, g*512:]."""
    nc = bacc.Bacc("TRN2", target_bir_lowering=False, debug=False)

    qt_d = nc.dram_tensor("qt", [P, KT, W], fp8, kind="ExternalInput")
    xt_d = nc.dram_tensor("xt", [P, KT, W], fp8, kind="ExternalInput")
    out_d = nc.dram_tensor("eout", [P, N_TILES, CHUNK], fp8, kind="ExternalOutput")

    with tile.TileContext(nc) as tc:
        with (
            tc.tile_pool(name="qp", bufs=1) as qp,
            tc.tile_pool(name="xp", bufs=1) as xp,
            tc.tile_pool(name="wp", bufs=1) as wp,
            tc.tile_pool(name="scrp", bufs=8) as scrp,
            tc.tile_pool(name="psp", bufs=2, space="PSUM") as psp,
        ):
            qt_sb = qp.tile([P, KT, W], fp8, tag="qt")
            xt_sb = xp.tile([P, KT, W], fp8, tag="xt")

            # PE warm-up: dummy matmuls on a zeroed tile while DMAs stream
            # (results land in PSUM and are never read).
            warm = wp.tile([P, 2, 640], fp8, tag="warm")
            nc.vector.memset(warm[:], 0)
            ps_w = psp.tile([P, CHUNK], f32, tag="ps")
            for _ in range(8):
                nc.tensor.matmul(
                    ps_w[:, 0:512], warm[:, :, 0:P], warm[:, :, P:640],
                    start=True, stop=True,
                    perf_mode=mybir.MatmulPerfMode.DoubleRow)

            # qt streams on the gpsimd queue, xt on the sync queue (the
            # output tiles share sync but lag far behind thanks to the
            # 8-deep scratch rotation); 1024-col chunks keep supply ahead
            # of the 512-cols-per-2us consumption.
            nc.gpsimd.dma_start(qt_sb[:, :, 0:CELL], qt_d[:, :, 0:CELL])
            nc.sync.dma_start(xt_sb[:, :, 0:CELL], xt_d[:, :, 0:CELL])
            for c0 in range(CELL, W, 1024):
                c1 = min(c0 + 1024, W)
                nc.gpsimd.dma_start(qt_sb[:, :, c0:c1], qt_d[:, :, c0:c1])
                nc.sync.dma_start(xt_sb[:, :, c0:c1], xt_d[:, :, c0:c1])

            for t in range(N_TILES):
                ps = psp.tile([P, CHUNK], f32, tag="ps")
                x0 = t * CELL
                for kt in range(2):
                    for g in range(4):
                        q0 = (4 * t + g) * P
                        nc.tensor.matmul(
                            ps[:, g * 512:(g + 1) * 512],
                            qt_sb[:, 2 * kt:2 * kt + 2, q0:q0 + P],
                            xt_sb[:, 2 * kt:2 * kt + 2, x0:x0 + CELL],
                            start=(kt == 0), stop=(kt == 1),
                            perf_mode=mybir.MatmulPerfMode.DoubleRow,
                        )
                scr = scrp.tile([P, CHUNK], fp8, tag="scr")
                with nc.allow_low_precision(reason="fp8 exp output is the design"):
                    nc.scalar.activation(scr[:], ps[:], AF.Exp, scale=INV_T)
                nc.sync.dma_start(out_d[:, t, :], scr[:])

    nc.finalize()
    return nc


def _l2n(x):
    n = np.sqrt(np.sum(x.astype(np.float32) ** 2, axis=1, keepdims=True))
    return x / np.maximum(n, 1e-12)


def _pack(z8):
    """[rows, 512] fp8 -> [128, 4, rows] SBUF layout: out[p,k,n] = z8[n, k*128+p]."""
    return np.ascontiguousarray(z8.T.reshape(KT, P, -1).transpose(1, 0, 2))


def prepare(emb_i, emb_j, emb_k):
    z_i = _l2n(emb_i)
    z_j = _l2n(emb_j)
    z_k = _l2n(emb_k)
    reps = np.concatenate([z_i, z_j], axis=0).astype(np.float32)  # [8192, 512]
    z8 = reps.astype(FP8_NP)
    z8f = z8.astype(np.float32)

    packed = _pack(z8)  # [128, 4, 8192]
    in_maps = []
    for c in range(8):
        tl = TILES[c * N_TILES:(c + 1) * N_TILES]
        qt = np.concatenate(
            [packed[:, :, i * CELL + s * P: i * CELL + (s + 1) * P]
             for _, units in tl for (i, s) in units], axis=2)
        xt = np.concatenate(
            [packed[:, :, j * CELL:(j + 1) * CELL] for j, _ in tl], axis=2)
        in_maps.append({"qt": np.ascontiguousarray(qt),
                        "xt": np.ascontiguousarray(xt)})

    pos = np.sum(z_i.astype(np.float64) * z_j.astype(np.float64), axis=1)
    sim_ik = np.sum(z_k.astype(np.float64) * z_i.astype(np.float64), axis=1)
    denom_fu = 2.0 * np.sum(np.exp(sim_ik * INV_T))
    diag = np.sum(z8f.astype(np.float64) * z8f.astype(np.float64), axis=1)
    self_term = np.exp(diag * INV_T).astype(FP8_NP).astype(np.float64)
    ctx = {"pos2": np.concatenate([pos, pos]), "denom_fu": denom_fu,
           "self_term": self_term}
    return in_maps, ctx


def assemble(results, ctx):
    """Row sums + symmetric column sums of the fp8 exp cells -> loss."""
    S = np.zeros(TWO_N, dtype=np.float64)
    for c, r in enumerate(results):
        e = np.asarray(r["eout"]).astype(np.float32)   # [128, 17, 2048]
        e4 = e.reshape(P, N_TILES, 4, CELL)
        rsum = e4.sum(axis=3, dtype=np.float64)        # [128, 17, 4]
        csum = e4.sum(axis=0, dtype=np.float64)        # [17, 4, 512]
        for t, (j, units) in enumerate(TILES[c * N_TILES:(c + 1) * N_TILES]):
            for g, (i, s) in enumerate(units):
                S[i * CELL + s * P:i * CELL + (s + 1) * P] += rsum[:, t, g]
                if i != j:
                    S[j * CELL:(j + 1) * CELL] += csum[t, g]
    denom = S - ctx["self_term"] + ctx["denom_fu"]
    loss = np.mean(np.log(denom) - INV_T * ctx["pos2"])
    return np.asarray(np.float32(loss))


_NC_CACHE = {}


def _get_nc():
    if "nc" not in _NC_CACHE:
        _NC_CACHE["nc"] = build_nc()
    return _NC_CACHE["nc"]


def kernel(emb_i, emb_j, emb_k):
    from concourse.bass_utils import run_bass_kernel_spmd

    in_maps, ctx = prepare(emb_i, emb_j, emb_k)
    nc = _get_nc()
    res = run_bass_kernel_spmd(nc, in_maps, list(range(8))).results
    return assemble(res, ctx)


# revision 12
# speedup vs baseline: 1.1740x; 1.1740x over previous
"""Trainium2 Bass kernel for nn_ContrastiveLoss (B=4096, D=512, 8 cores).

Strategy v5 (symmetric-triangle, run-compressed inputs):
  - Host l2-normalizes [emb_i; emb_j] -> reps [8192, 512] (fp32), quantizes
    to fp8 e4m3, and packs per-core SBUF layouts.
  - The 8192x8192 similarity matrix is a 16x16 grid of 512x512 cells; only
    the 136 upper-triangle cells are computed (sim is symmetric).  One
    device tile [128 x 2048] = one cell: 4 row sub-slices (slots) x the
    cell's 512-column block.  136 cells = 17 per core, perfectly balanced.
  - Per-core tiles follow a fixed run pattern over x-blocks,
    [3,3,3,3,3,1,1]: the multiset of column sizes {1..16} cuts exactly
    into 40 three-cell pieces + 16 one-cell pieces = 8 cores x (5 threes +
    2 ones).  So each core loads only 7 x-blocks (1.84 MB) instead of one
    per tile, plus 17 q-blocks (4.46 MB) -- DMA was the binding resource.
  - Per tile: fp8 DoubleRow matmuls (K=256/instr) -> 4-bank PSUM, ScalarE
    exp(5x) reading PSUM directly, fp8 result DMAed out on the sync queue
    (inputs stream on gpsimd).  Dummy warm-up matmuls hold the PE HAM
    clock-gate at 8/8 until real work arrives.
  - Host reduces: row sums of each cell feed its grid-row block, column
    sums of off-diagonal cells feed the transposed block, then the known
    fp8 diagonal term is subtracted and the host-computed fu scalar and
    positive-pair terms are added.

exp(5*sim) lies in [e^-1.5, e^5] ~ [0.2, 149]: inside fp8 e4m3 normal
range.  fp8 quantization of inputs+outputs yields ~5e-5 final loss error
vs the 2e-2 gate.
"""

import numpy as np

import concourse.bass as bass
import concourse.mybir as mybir
import concourse.tile as tile
from concourse import bacc

f32 = mybir.dt.float32
fp8 = mybir.dt.float8e4
AF = mybir.ActivationFunctionType

P = 128
TEMP = 0.2
INV_T = 1.0 / TEMP  # 5.0
FP8_NP = mybir.dt.np(fp8)

B, D = 4096, 512
TWO_N = 2 * B           # 8192
KT = D // P             # 4 k-subtiles of 128
CHUNK = 2048            # ACT/psum tile free size (4 PSUM banks)
CELL = 512              # grid cell edge
G = TWO_N // CELL       # 16x16 grid
N_TILES = 17            # tiles (cells) per core
RUNS = [3, 3, 3, 3, 3, 1, 1]   # tiles per x-block slot
N_XB = len(RUNS)        # 7 x-block slots per core
QW = N_TILES * CELL     # 8704
XW = N_XB * CELL        # 3584
# x-slot of tile t under RUNS
XSLOT = [s for s, r in enumerate(RUNS) for _ in range(r)]


def _pieces():
    """Cut the 136 upper-triangle cells into 40 three-cell and 16 one-cell
    pieces, each piece within one grid column."""
    threes, ones = [], []
    for j in range(G):
        cells = [(i, j) for i in range(j + 1)]
        for _ in range((j + 1) % 3):
            ones.append([cells.pop(0)])
        for k in range(0, len(cells), 3):
            threes.append(cells[k:k + 3])
    assert len(threes) == 40 and len(ones) == 16
    return threes, ones


def _core_cells():
    """Per-core list of 17 cells, ordered to match the RUNS pattern."""
    threes, ones = _pieces()
    cores = []
    for c in range(8):
        pieces = threes[c * 5:(c + 1) * 5] + ones[c * 2:(c + 1) * 2]
        cores.append([cell for p in pieces for cell in p])
    return cores


CORE_CELLS = _core_cells()


def build_nc():
    """SPMD program: tile t (= cell) computes, for slot g in 0..3,
    exp(5 * q[t*512+g*128 :][128] @ x[XSLOT[t]*512 :][512]) -> eout[:, t, g*512:]."""
    nc = bacc.Bacc("TRN2", target_bir_lowering=False, debug=False)

    qt_d = nc.dram_tensor("qt", [P, KT, QW], fp8, kind="ExternalInput")
    xt_d = nc.dram_tensor("xt", [P, KT, XW], fp8, kind="ExternalInput")
    out_d = nc.dram_tensor("eout", [P, N_TILES, CHUNK], fp8, kind="ExternalOutput")

    with tile.TileContext(nc) as tc:
        with (
            tc.tile_pool(name="qp", bufs=1) as qp,
            tc.tile_pool(name="xp", bufs=1) as xp,
            tc.tile_pool(name="wp", bufs=1) as wp,
            tc.tile_pool(name="scrp", bufs=8) as scrp,
            tc.tile_pool(name="psp", bufs=2, space="PSUM") as psp,
        ):
            qt_sb = qp.tile([P, KT, QW], fp8, tag="qt")
            xt_sb = xp.tile([P, KT, XW], fp8, tag="xt")

            # PE warm-up: dummy matmuls on a zeroed tile while DMAs stream
            # (results land in PSUM and are never read).
            warm = wp.tile([P, 2, 640], fp8, tag="warm")
            nc.vector.memset(warm[:], 0)
            ps_w = psp.tile([P, CHUNK], f32, tag="ps")
            for _ in range(8):
                nc.tensor.matmul(
                    ps_w[:, 0:512], warm[:, :, 0:P], warm[:, :, P:640],
                    start=True, stop=True,
                    perf_mode=mybir.MatmulPerfMode.DoubleRow)

            # inputs on the gpsimd queue, interleaved in consumption order
            # (x-block for a run just before the q-blocks of its tiles);
            # outputs go on the sync queue.
            for t in range(N_TILES):
                if t == 0 or XSLOT[t] != XSLOT[t - 1]:
                    s = XSLOT[t]
                    nc.gpsimd.dma_start(
                        xt_sb[:, :, s * CELL:(s + 1) * CELL],
                        xt_d[:, :, s * CELL:(s + 1) * CELL])
                nc.gpsimd.dma_start(
                    qt_sb[:, :, t * CELL:(t + 1) * CELL],
                    qt_d[:, :, t * CELL:(t + 1) * CELL])

            for t in range(N_TILES):
                ps = psp.tile([P, CHUNK], f32, tag="ps")
                x0 = XSLOT[t] * CELL
                for kt in range(2):
                    for g in range(4):
                        q0 = t * CELL + g * P
                        nc.tensor.matmul(
                            ps[:, g * 512:(g + 1) * 512],
                            qt_sb[:, 2 * kt:2 * kt + 2, q0:q0 + P],
                            xt_sb[:, 2 * kt:2 * kt + 2, x0:x0 + CELL],
                            start=(kt == 0), stop=(kt == 1),
                            perf_mode=mybir.MatmulPerfMode.DoubleRow,
                        )
                scr = scrp.tile([P, CHUNK], fp8, tag="scr")
                with nc.allow_low_precision(reason="fp8 exp output is the design"):
                    nc.scalar.activation(scr[:], ps[:], AF.Exp, scale=INV_T)
                nc.sync.dma_start(out_d[:, t, :], scr[:])

    nc.finalize()
    return nc


def _l2n(x):
    n = np.sqrt(np.sum(x.astype(np.float32) ** 2, axis=1, keepdims=True))
    return x / np.maximum(n, 1e-12)


def _pack(z8):
    """[rows, 512] fp8 -> [128, 4, rows] SBUF layout: out[p,k,n] = z8[n, k*128+p]."""
    return np.ascontiguousarray(z8.T.reshape(KT, P, -1).transpose(1, 0, 2))


def prepare(emb_i, emb_j, emb_k):
    z_i = _l2n(emb_i)
    z_j = _l2n(emb_j)
    z_k = _l2n(emb_k)
    reps = np.concatenate([z_i, z_j], axis=0).astype(np.float32)  # [8192, 512]
    z8 = reps.astype(FP8_NP)
    z8f = z8.astype(np.float32)

    packed = _pack(z8)  # [128, 4, 8192]
    blk = [packed[:, :, b * CELL:(b + 1) * CELL] for b in range(G)]
    in_maps = []
    for c in range(8):
        cells = CORE_CELLS[c]
        qt = np.concatenate([blk[i] for i, _ in cells], axis=2)
        xb = []
        for t, (_, j) in enumerate(cells):
            if t == 0 or XSLOT[t] != XSLOT[t - 1]:
                xb.append(blk[j])
        xt = np.concatenate(xb, axis=2)
        in_maps.append({"qt": np.ascontiguousarray(qt),
                        "xt": np.ascontiguousarray(xt)})

    pos = np.sum(z_i.astype(np.float64) * z_j.astype(np.float64), axis=1)
    sim_ik = np.sum(z_k.astype(np.float64) * z_i.astype(np.float64), axis=1)
    denom_fu = 2.0 * np.sum(np.exp(sim_ik * INV_T))
    diag = np.sum(z8f.astype(np.float64) * z8f.astype(np.float64), axis=1)
    self_term = np.exp(diag * INV_T).astype(FP8_NP).astype(np.float64)
    ctx = {"pos2": np.concatenate([pos, pos]), "denom_fu": denom_fu,
           "self_term": self_term}
    return in_maps, ctx


def assemble(results, ctx):
    """Row sums + symmetric column sums of the fp8 exp cells -> loss."""
    S = np.zeros(TWO_N, dtype=np.float64)
    for c, r in enumerate(results):
        e = np.asarray(r["eout"]).astype(np.float32)   # [128, 17, 2048]
        e4 = e.reshape(P, N_TILES, 4, CELL)
        rsum = e4.sum(axis=3, dtype=np.float64)        # [128, 17, 4]
        csum = e4.sum(axis=0, dtype=np.float64)        # [17, 4, 512]
        for t, (i, j) in enumerate(CORE_CELLS[c]):
            for g in range(4):
                S[i * CELL + g * P:i * CELL + (g + 1) * P] += rsum[:, t, g]
            if i != j:
                S[j * CELL:(j + 1) * CELL] += csum[t].sum(axis=0)
    denom = S - ctx["self_term"] + ctx["denom_fu"]
    loss = np.mean(np.log(denom) - INV_T * ctx["pos2"])
    return np.asarray(np.float32(loss))


_NC_CACHE = {}


def _get_nc():
    if "nc" not in _NC_CACHE:
        _NC_CACHE["nc"] = build_nc()
    return _NC_CACHE["nc"]


def kernel(emb_i, emb_j, emb_k):
    from concourse.bass_utils import run_bass_kernel_spmd

    in_maps, ctx = prepare(emb_i, emb_j, emb_k)
    nc = _get_nc()
    res = run_bass_kernel_spmd(nc, in_maps, list(range(8))).results
    return assemble(res, ctx)
